# revision 28
# baseline (speedup 1.0000x reference)
"""Trainium2 Bass kernel for DifferentiableSparseHypergraph (topk_masking).

Full computation per batch n:
  x_mean = x[n].mean(T)                      (C, V)
  q = Wq @ x_mean + bq                       (O=32, V)   [1x1 conv == matmul]
  q = q / max(||q||_2 over O, eps)
  H_raw = (q^T @ key_prototypes) / sqrt(O)   (V, M=128)
  topk10 -> softmax over the 10 vals -> scatter back; zeros elsewhere.

Kernel strategy (pure data-parallel over batch, 8 cores x 8 batches):
  * mean-over-T and the 1x1 conv are fused into PSUM-accumulated matmuls:
    psum[o, tl*64+v] += sum_c WqT[c,o] * x[c, t=8g+tl, v], accumulated over
    the 2 c-halves and 8 t-groups g => a final 8-way free-dim reduce gives
    sum_t (Wq @ x[:, t, :]).
  * L2 norm over channels is computed with a ones-matmul (partition-dim
    reduction on the PE), rsqrt on ACT+DVE.
  * top-10 per row is index-free: t_k = 10th largest per row (via the DVE
    max/match_replace/max top-8 primitives), and the output is
    exp(H) * (H >= t_k) / sum(exp(H) * (H >= t_k))  -- identical to
    softmax-over-topk scattered back (softmax is shift/subset invariant).
"""

import numpy as np

import concourse.bacc as bacc
import concourse.bass as bass
import concourse.mybir as mybir
import concourse.tile as tile

N, C, T, V = 64, 256, 64, 64
INTER = 32          # conv out channels
M = 128             # num hyperedges
TOPK = 10
NCORES = 8
FP = mybir.dt.float32
NEG_BIG = -1.0e30


def build_nc(nloc: int) -> bass.Bass:
    """Build the per-core Bass program processing `nloc` batches."""
    assert nloc % 2 == 0
    npair = nloc // 2
    # Bacc (not bare Bass): its compile()/finalize() pipeline splits
    # multi-semaphore waits into InstEventSemaphore pairs — walrus allows
    # at most one sync wait per regular instruction.
    nc = bacc.Bacc(target_bir_lowering=False, debug=False)

    x = nc.dram_tensor("x", (nloc, C, T, V), FP, kind="ExternalInput")
    wqt = nc.dram_tensor("wqt", (C, INTER), FP, kind="ExternalInput")
    kp = nc.dram_tensor("kp", (INTER, M), FP, kind="ExternalInput")
    bq = nc.dram_tensor("bq", (INTER, 1), FP, kind="ExternalInput")
    out = nc.dram_tensor("out", (nloc, V, M), FP, kind="ExternalOutput")

    A = mybir.AluOpType
    AF = mybir.ActivationFunctionType
    from concourse.tile import add_dep_helper

    with tile.TileContext(nc) as tc:
        with (
            tc.tile_pool(name="consts", bufs=1) as consts,
            tc.tile_pool(name="xph", bufs=2) as xph,
            tc.tile_pool(name="xq", bufs=1) as xq,
            tc.tile_pool(name="xp", bufs=2) as xp,
            tc.tile_pool(name="small", bufs=2) as small,
            tc.tile_pool(name="psA", bufs=2, space="PSUM") as psA,
            tc.tile_pool(name="psB", bufs=2, space="PSUM") as psB,
            tc.tile_pool(name="psS", bufs=1, space="PSUM") as psS,
        ):
            # --- batch 0's x loads go FIRST (before the const DMAs) and in
            # 1 MiB t-range chunks, so the DVE tree starts as early as
            # possible.  Batch 0's first level pairs t with t+1 (each chunk
            # is self-contained); later batches pair t with t+32.
            xh0 = [
                xph.tile([128, T * V], FP, tag=f"xh{h}", name=f"xh0_{h}")
                for h in range(2)
            ]
            for h in range(2):
                for c in range(2):
                    eng = nc.sync if h == 0 else nc.scalar
                    eng.dma_start(
                        out=xh0[h][:, c * 2048 : (c + 1) * 2048],
                        in_=x[0, h * 128 : (h + 1) * 128,
                              c * (T // 2) : (c + 1) * (T // 2)],
                    )
            # prefetch the LAST batch now: its data is resident long before
            # the end, so the kernel tail isn't gated on the final DMA
            xh_last = [
                xq.tile([128, T * V], FP, tag=f"xl{h}", name=f"xhl_{h}")
                for h in range(2)
            ]
            if nloc > 1:
                for h in range(2):
                    eng = nc.sync if h == 0 else nc.scalar
                    eng.dma_start(
                        out=xh_last[h][:],
                        in_=x[nloc - 1, h * 128 : (h + 1) * 128],
                    )

            # --- replicated constants ---
            wq_sb = consts.tile([128, 2, INTER], FP)    # [c, c_half, o]
            nc.sync.dma_start(
                out=wq_sb[:], in_=wqt.rearrange("(h c) o -> c h o", h=2)
            )
            kp_sb = consts.tile([INTER, M], FP)
            nc.sync.dma_start(out=kp_sb[:], in_=kp[:])
            bq_sb = consts.tile([INTER, 1], FP)
            nc.sync.dma_start(out=bq_sb[:], in_=bq[:])
            ones_sb = consts.tile([INTER, 1], FP)
            nc.vector.memset(ones_sb[:], 1.0)

            # The fp32 self-loading matmul can carry at most ONE semaphore
            # wait (walrus S3_LW_STRUCT limit). Absorb the wq/kp DMA waits
            # with dummy 1x1 matmuls so the first real matmuls only wait on
            # their x-tile DMA.
            scr = psS.tile([1, 1], FP)
            d1 = nc.tensor.matmul(
                scr[:], wq_sb[:, 0, 0:1], wq_sb[:, 0, 0:1], start=True, stop=True
            )
            d2 = nc.tensor.matmul(
                scr[:], kp_sb[:, 0:1], kp_sb[:, 0:1], start=True, stop=True
            )
            add_dep_helper(d2.ins, d1.ins, sync=False, reason="pe-wait-absorb order")

            q2 = None
            first_mm = None
            for n in range(nloc):
                if n == 0:
                    xh = xh0
                elif n == nloc - 1:
                    xh = xh_last
                else:
                    xh = []
                    for h in range(2):
                        t = xph.tile([128, T * V], FP, tag=f"xh{h}")
                        eng = nc.sync if h == 0 else nc.scalar
                        eng.dma_start(
                            out=t[:], in_=x[n, h * 128 : (h + 1) * 128]
                        )
                        xh.append(t)

                # t-axis tree reduction on DVE: t 64 -> 32 -> 16 -> 8.
                # (DVE only: concurrent GPSIMD tensor work halves DVE
                # throughput via SBUF port contention — measured.)
                r3 = []
                for h in range(2):
                    a1 = xp.tile([128, T * V // 2], FP, tag=f"a1{h}")
                    if n == 0:
                        # pair t,t+1 chunk-locally so each chunk's add can
                        # run as soon as its 1 MiB DMA lands
                        for c in range(2):
                            src = xh[h][
                                :, c * 2048 : (c + 1) * 2048
                            ].rearrange("p (t two v) -> p t two v", two=2, v=V)
                            dst = a1[
                                :, c * 1024 : (c + 1) * 1024
                            ].rearrange("p (t v) -> p t v", v=V)
                            nc.vector.tensor_add(
                                dst, src[:, :, 0, :], src[:, :, 1, :]
                            )
                    else:
                        nc.vector.tensor_add(
                            a1[:],
                            xh[h][:, : T * V // 2],
                            xh[h][:, T * V // 2 :],
                        )
                    a2 = xp.tile([128, T * V // 4], FP, tag=f"a2{h}")
                    nc.vector.tensor_add(
                        a2[:], a1[:, : T * V // 4], a1[:, T * V // 4 :]
                    )
                    a3 = xp.tile([128, T * V // 8], FP, tag=f"a3{h}")
                    nc.vector.tensor_add(
                        a3[:], a2[:, : T * V // 8], a2[:, T * V // 8 :]
                    )
                    r3.append(a3)

                # fused rest-of-mean + conv: accumulate c-halves into one
                # psum group; psum free = (tl, v) partial t-sums
                l = n % 2
                m = n
                if l == 0:
                    q2 = small.tile([INTER, 2 * V], FP, tag="q2")
                pa = psA.tile([INTER, 512], FP, tag="pa")
                for h in range(2):
                    mm = nc.tensor.matmul(
                        pa[:],
                        wq_sb[:, h, :],
                        r3[h][:],
                        start=(h == 0),
                        stop=(h == 1),
                    )
                    if first_mm is None:
                        first_mm = mm
                        add_dep_helper(
                            mm.ins, d2.ins, sync=False,
                            reason="pe-wait-absorb order",
                        )
                qtmp = small.tile([INTER, V], FP, tag="qtmp")
                nc.vector.reduce_sum(
                    out=qtmp[:],
                    in_=pa[:].rearrange("o (t v) -> o v t", t=8),
                    axis=mybir.AxisListType.X,
                )
                # q = qsum/T + bq on the idle ACT engine
                nc.scalar.activation(
                    q2[:, l * V : (l + 1) * V],
                    qtmp[:],
                    AF.Identity,
                    bias=bq_sb[:],
                    scale=1.0 / T,
                )
                if l == 0:
                    continue
                p = m // 2

                # scores: G[vv, m] = q2 . kp (unnormalized); the top-k
                # SELECTION is invariant to the positive per-row norm scale,
                # so it runs on G in parallel with the norm computation, and
                # the normalization happens inside the Exp (scale AP).
                qsq = small.tile([INTER, 2 * V], FP, tag="qsq")
                nc.vector.tensor_mul(qsq[:], q2[:], q2[:])
                pb = psB.tile([2 * V, M], FP, tag="pb")
                nc.tensor.matmul(pb[:], q2[:], kp_sb[:], start=True, stop=True)
                pc = psB.tile([2 * V, 1], FP, tag="pc")
                nc.tensor.matmul(pc[:], qsq[:], ones_sb[:], start=True, stop=True)
                gsb = small.tile([2 * V, M], FP, tag="gsb")
                nc.scalar.activation(gsb[:], pb[:], AF.Copy)

                # rn = 1/sqrt(INTER * nsq) = INTER^-0.5 / ||q||
                nrm = small.tile([2 * V, 1], FP, tag="nrm")
                nc.scalar.activation(nrm[:], pc[:], AF.Sqrt, scale=float(INTER))
                rn = small.tile([2 * V, 1], FP, tag="rn")
                nc.vector.reciprocal(rn[:], nrm[:])

                # tG_k = 10th largest of G per row: top8, knock out, top8
                top8a = small.tile([2 * V, 8], FP, tag="t8a")
                nc.vector.max(top8a[:], gsb[:])
                work = small.tile([2 * V, M], FP, tag="work")
                nc.vector.match_replace(work[:], top8a[:], gsb[:], NEG_BIG)
                top8b = small.tile([2 * V, 8], FP, tag="t8b")
                nc.vector.max(top8b[:], work[:])

                # masked softmax without scatter:
                # e = exp(G * rn); me = (G >= tG_k) * e; out = me / sum(me)
                e = small.tile([2 * V, M], FP, tag="e")
                nc.scalar.activation(e[:], pb[:], AF.Exp, scale=rn[:])
                me = small.tile([2 * V, M], FP, tag="me")
                s = small.tile([2 * V, 1], FP, tag="s")
                nc.vector.scalar_tensor_tensor(
                    out=me[:],
                    in0=gsb[:],
                    scalar=top8b[:, 1:2],
                    in1=e[:],
                    op0=A.is_ge,
                    op1=A.mult,
                    accum_out=s[:],
                )
                r = small.tile([2 * V, 1], FP, tag="r")
                nc.vector.reciprocal(r[:], s[:])
                ot = small.tile([2 * V, M], FP, tag="ot")
                nc.scalar.activation(ot[:], me[:], AF.Copy, scale=r[:])

                nc.scalar.dma_start(
                    out=out[2 * p : 2 * p + 2].rearrange("b v m -> (b v) m"),
                    in_=ot[:],
                )
    nc.finalize()
    return nc


_NC_CACHE: dict[int, bass.Bass] = {}


def _get_nc(nloc: int) -> bass.Bass:
    if nloc not in _NC_CACHE:
        _NC_CACHE[nloc] = build_nc(nloc)
    return _NC_CACHE[nloc]


def _make_in_maps(x, Wq, bq, key_prototypes, ncores):
    nloc = x.shape[0] // ncores
    wqt = np.ascontiguousarray(np.asarray(Wq, dtype=np.float32).T)
    kpc = np.ascontiguousarray(np.asarray(key_prototypes, dtype=np.float32))
    bqc = np.ascontiguousarray(
        np.asarray(bq, dtype=np.float32).reshape(INTER, 1)
    )
    xc = np.asarray(x, dtype=np.float32)
    return [
        {
            "x": np.ascontiguousarray(xc[i * nloc : (i + 1) * nloc]),
            "wqt": wqt,
            "kp": kpc,
            "bq": bqc,
        }
        for i in range(ncores)
    ]


def run(inputs, trace: bool = False):
    """Run on hardware; returns (full_output, BassKernelResults)."""
    from concourse.bass_utils import run_bass_kernel_spmd

    x = inputs["x"]
    nloc = x.shape[0] // NCORES
    nc = _get_nc(nloc)
    in_maps = _make_in_maps(
        x, inputs["Wq"], inputs["bq"], inputs["key_prototypes"], NCORES
    )
    res = run_bass_kernel_spmd(nc, in_maps, list(range(NCORES)), trace=trace)
    out = np.concatenate([r["out"] for r in res.results], axis=0)
    return out, res


def kernel(**inputs) -> np.ndarray:
    out, _ = run(inputs, trace=False)
    return out


# revision 29
# speedup vs baseline: 1.0759x; 1.0759x over previous
"""Trainium2 Bass kernel for DifferentiableSparseHypergraph (topk_masking).

Full computation per batch n:
  x_mean = x[n].mean(T)                      (C, V)
  q = Wq @ x_mean + bq                       (O=32, V)   [1x1 conv == matmul]
  q = q / max(||q||_2 over O, eps)
  H_raw = (q^T @ key_prototypes) / sqrt(O)   (V, M=128)
  topk10 -> softmax over the 10 vals -> scatter back; zeros elsewhere.

Kernel strategy (pure data-parallel over batch, 8 cores x 8 batches):
  * mean-over-T and the 1x1 conv are fused into PSUM-accumulated matmuls:
    psum[o, tl*64+v] += sum_c WqT[c,o] * x[c, t=8g+tl, v], accumulated over
    the 2 c-halves and 8 t-groups g => a final 8-way free-dim reduce gives
    sum_t (Wq @ x[:, t, :]).
  * L2 norm over channels is computed with a ones-matmul (partition-dim
    reduction on the PE), rsqrt on ACT+DVE.
  * top-10 per row is index-free: t_k = 10th largest per row (via the DVE
    max/match_replace/max top-8 primitives), and the output is
    exp(H) * (H >= t_k) / sum(exp(H) * (H >= t_k))  -- identical to
    softmax-over-topk scattered back (softmax is shift/subset invariant).
"""

import numpy as np

import concourse.bacc as bacc
import concourse.bass as bass
import concourse.mybir as mybir
import concourse.tile as tile

N, C, T, V = 64, 256, 64, 64
INTER = 32          # conv out channels
M = 128             # num hyperedges
TOPK = 10
NCORES = 8
FP = mybir.dt.float32
NEG_BIG = -1.0e30


def build_nc(nloc: int) -> bass.Bass:
    """Build the per-core Bass program processing `nloc` batches."""
    assert nloc % 2 == 0
    npair = nloc // 2
    # Bacc (not bare Bass): its compile()/finalize() pipeline splits
    # multi-semaphore waits into InstEventSemaphore pairs — walrus allows
    # at most one sync wait per regular instruction.
    nc = bacc.Bacc(target_bir_lowering=False, debug=False)

    x = nc.dram_tensor("x", (nloc, C, T, V), FP, kind="ExternalInput")
    wqt = nc.dram_tensor("wqt", (C, INTER), FP, kind="ExternalInput")
    kp = nc.dram_tensor("kp", (INTER, M), FP, kind="ExternalInput")
    bq = nc.dram_tensor("bq", (INTER, 1), FP, kind="ExternalInput")
    out = nc.dram_tensor("out", (nloc, V, M), FP, kind="ExternalOutput")

    A = mybir.AluOpType
    AF = mybir.ActivationFunctionType
    from concourse.tile import add_dep_helper

    with tile.TileContext(nc) as tc:
        with (
            tc.tile_pool(name="consts", bufs=1) as consts,
            tc.tile_pool(name="xph", bufs=3) as xph,
            tc.tile_pool(name="xp", bufs=2) as xp,
            tc.tile_pool(name="small", bufs=2) as small,
            tc.tile_pool(name="psA", bufs=2, space="PSUM") as psA,
            tc.tile_pool(name="psB", bufs=2, space="PSUM") as psB,
            tc.tile_pool(name="psS", bufs=1, space="PSUM") as psS,
        ):
            # --- batch 0's x loads go FIRST (before the const DMAs) and in
            # 1 MiB t-range chunks, so the DVE tree starts as early as
            # possible.  Batch 0's first level pairs t with t+1 (each chunk
            # is self-contained); later batches pair t with t+32.
            xh0 = [
                xph.tile([128, T * V], FP, tag=f"xh{h}", name=f"xh0_{h}")
                for h in range(2)
            ]
            for h in range(2):
                for c in range(2):
                    nc.sync.dma_start(
                        out=xh0[h][:, c * 2048 : (c + 1) * 2048],
                        in_=x[0, h * 128 : (h + 1) * 128,
                              c * (T // 2) : (c + 1) * (T // 2)],
                    )

            # --- replicated constants ---
            wq_sb = consts.tile([128, 2, INTER], FP)    # [c, c_half, o]
            nc.sync.dma_start(
                out=wq_sb[:], in_=wqt.rearrange("(h c) o -> c h o", h=2)
            )
            kp_sb = consts.tile([INTER, M], FP)
            nc.sync.dma_start(out=kp_sb[:], in_=kp[:])
            bq_sb = consts.tile([INTER, 1], FP)
            nc.sync.dma_start(out=bq_sb[:], in_=bq[:])
            ones_sb = consts.tile([INTER, 1], FP)
            nc.vector.memset(ones_sb[:], 1.0)

            # The fp32 self-loading matmul can carry at most ONE semaphore
            # wait (walrus S3_LW_STRUCT limit). Absorb the wq/kp DMA waits
            # with dummy 1x1 matmuls so the first real matmuls only wait on
            # their x-tile DMA.
            scr = psS.tile([1, 1], FP)
            d1 = nc.tensor.matmul(
                scr[:], wq_sb[:, 0, 0:1], wq_sb[:, 0, 0:1], start=True, stop=True
            )
            d2 = nc.tensor.matmul(
                scr[:], kp_sb[:, 0:1], kp_sb[:, 0:1], start=True, stop=True
            )
            add_dep_helper(d2.ins, d1.ins, sync=False, reason="pe-wait-absorb order")

            q2 = None
            first_mm = None
            for n in range(nloc):
                if n == 0:
                    xh = xh0
                else:
                    xh = []
                    for h in range(2):
                        t = xph.tile([128, T * V], FP, tag=f"xh{h}")
                        nc.sync.dma_start(
                            out=t[:], in_=x[n, h * 128 : (h + 1) * 128]
                        )
                        xh.append(t)

                # t-axis tree reduction on DVE: t 64 -> 32 -> 16 -> 8.
                # (DVE only: concurrent GPSIMD tensor work halves DVE
                # throughput via SBUF port contention — measured.)
                r3 = []
                for h in range(2):
                    a1 = xp.tile([128, T * V // 2], FP, tag=f"a1{h}")
                    if n == 0:
                        # pair t,t+1 chunk-locally so each chunk's add can
                        # run as soon as its 1 MiB DMA lands
                        for c in range(2):
                            src = xh[h][
                                :, c * 2048 : (c + 1) * 2048
                            ].rearrange("p (t two v) -> p t two v", two=2, v=V)
                            dst = a1[
                                :, c * 1024 : (c + 1) * 1024
                            ].rearrange("p (t v) -> p t v", v=V)
                            nc.vector.tensor_add(
                                dst, src[:, :, 0, :], src[:, :, 1, :]
                            )
                    else:
                        nc.vector.tensor_add(
                            a1[:],
                            xh[h][:, : T * V // 2],
                            xh[h][:, T * V // 2 :],
                        )
                    a2 = xp.tile([128, T * V // 4], FP, tag=f"a2{h}")
                    nc.vector.tensor_add(
                        a2[:], a1[:, : T * V // 4], a1[:, T * V // 4 :]
                    )
                    a3 = xp.tile([128, T * V // 8], FP, tag=f"a3{h}")
                    nc.vector.tensor_add(
                        a3[:], a2[:, : T * V // 8], a2[:, T * V // 8 :]
                    )
                    r3.append(a3)

                # fused rest-of-mean + conv: accumulate c-halves into one
                # psum group; psum free = (tl, v) partial t-sums
                l = n % 2
                m = n
                if l == 0:
                    q2 = small.tile([INTER, 2 * V], FP, tag="q2")
                pa = psA.tile([INTER, 512], FP, tag="pa")
                for h in range(2):
                    mm = nc.tensor.matmul(
                        pa[:],
                        wq_sb[:, h, :],
                        r3[h][:],
                        start=(h == 0),
                        stop=(h == 1),
                    )
                    if first_mm is None:
                        first_mm = mm
                        add_dep_helper(
                            mm.ins, d2.ins, sync=False,
                            reason="pe-wait-absorb order",
                        )
                qtmp = small.tile([INTER, V], FP, tag="qtmp")
                nc.vector.reduce_sum(
                    out=qtmp[:],
                    in_=pa[:].rearrange("o (t v) -> o v t", t=8),
                    axis=mybir.AxisListType.X,
                )
                # q = qsum/T + bq on the idle ACT engine
                nc.scalar.activation(
                    q2[:, l * V : (l + 1) * V],
                    qtmp[:],
                    AF.Identity,
                    bias=bq_sb[:],
                    scale=1.0 / T,
                )
                if l == 0:
                    continue
                p = m // 2

                # scores: G[vv, m] = q2 . kp (unnormalized); the top-k
                # SELECTION is invariant to the positive per-row norm scale,
                # so it runs on G in parallel with the norm computation, and
                # the normalization happens inside the Exp (scale AP).
                qsq = small.tile([INTER, 2 * V], FP, tag="qsq")
                nc.vector.tensor_mul(qsq[:], q2[:], q2[:])
                pb = psB.tile([2 * V, M], FP, tag="pb")
                nc.tensor.matmul(pb[:], q2[:], kp_sb[:], start=True, stop=True)
                pc = psB.tile([2 * V, 1], FP, tag="pc")
                nc.tensor.matmul(pc[:], qsq[:], ones_sb[:], start=True, stop=True)
                gsb = small.tile([2 * V, M], FP, tag="gsb")
                nc.scalar.activation(gsb[:], pb[:], AF.Copy)

                # rn = 1/sqrt(INTER * nsq) = INTER^-0.5 / ||q||
                nrm = small.tile([2 * V, 1], FP, tag="nrm")
                nc.scalar.activation(nrm[:], pc[:], AF.Sqrt, scale=float(INTER))
                rn = small.tile([2 * V, 1], FP, tag="rn")
                nc.vector.reciprocal(rn[:], nrm[:])

                # tG_k = 10th largest of G per row: top8, knock out, top8
                top8a = small.tile([2 * V, 8], FP, tag="t8a")
                nc.vector.max(top8a[:], gsb[:])
                work = small.tile([2 * V, M], FP, tag="work")
                nc.vector.match_replace(work[:], top8a[:], gsb[:], NEG_BIG)
                top8b = small.tile([2 * V, 8], FP, tag="t8b")
                nc.vector.max(top8b[:], work[:])

                # masked softmax without scatter:
                # e = exp(G * rn); me = (G >= tG_k) * e; out = me / sum(me)
                e = small.tile([2 * V, M], FP, tag="e")
                nc.scalar.activation(e[:], pb[:], AF.Exp, scale=rn[:])
                me = small.tile([2 * V, M], FP, tag="me")
                s = small.tile([2 * V, 1], FP, tag="s")
                nc.vector.scalar_tensor_tensor(
                    out=me[:],
                    in0=gsb[:],
                    scalar=top8b[:, 1:2],
                    in1=e[:],
                    op0=A.is_ge,
                    op1=A.mult,
                    accum_out=s[:],
                )
                r = small.tile([2 * V, 1], FP, tag="r")
                nc.vector.reciprocal(r[:], s[:])
                ot = small.tile([2 * V, M], FP, tag="ot")
                nc.scalar.activation(ot[:], me[:], AF.Copy, scale=r[:])

                nc.sync.dma_start(
                    out=out[2 * p : 2 * p + 2].rearrange("b v m -> (b v) m"),
                    in_=ot[:],
                )
    nc.finalize()
    return nc


_NC_CACHE: dict[int, bass.Bass] = {}


def _get_nc(nloc: int) -> bass.Bass:
    if nloc not in _NC_CACHE:
        _NC_CACHE[nloc] = build_nc(nloc)
    return _NC_CACHE[nloc]


def _make_in_maps(x, Wq, bq, key_prototypes, ncores):
    nloc = x.shape[0] // ncores
    wqt = np.ascontiguousarray(np.asarray(Wq, dtype=np.float32).T)
    kpc = np.ascontiguousarray(np.asarray(key_prototypes, dtype=np.float32))
    bqc = np.ascontiguousarray(
        np.asarray(bq, dtype=np.float32).reshape(INTER, 1)
    )
    xc = np.asarray(x, dtype=np.float32)
    return [
        {
            "x": np.ascontiguousarray(xc[i * nloc : (i + 1) * nloc]),
            "wqt": wqt,
            "kp": kpc,
            "bq": bqc,
        }
        for i in range(ncores)
    ]


def run(inputs, trace: bool = False):
    """Run on hardware; returns (full_output, BassKernelResults)."""
    from concourse.bass_utils import run_bass_kernel_spmd

    x = inputs["x"]
    nloc = x.shape[0] // NCORES
    nc = _get_nc(nloc)
    in_maps = _make_in_maps(
        x, inputs["Wq"], inputs["bq"], inputs["key_prototypes"], NCORES
    )
    res = run_bass_kernel_spmd(nc, in_maps, list(range(NCORES)), trace=trace)
    out = np.concatenate([r["out"] for r in res.results], axis=0)
    return out, res


def kernel(**inputs) -> np.ndarray:
    out, _ = run(inputs, trace=False)
    return out
